# revision 7
# baseline (speedup 1.0000x reference)
"""Joint Maximum Mean Discrepancy loss on 8 Trainium2 NeuronCores.

Math: for streams (s0,t0) and (s1,t1), the reference builds per-stream
Gaussian kernels K_r = exp(-gamma_r * dist_r) over feats_r = [src; tgt]
(N=8192 rows), takes their elementwise product, and returns
mean(s2s + t2t - 2*s2t) over the B x B blocks.

Device decomposition:
  exponent E_ij = 2*(W @ W.T)_ij - c_i - c_j
  with W = [sqrt(g0)*X0, sqrt(g1)*X1] (N x 320), c_i = g0*|X0_i|^2 +
  g1*|X1_i|^2, and gamma_r from the closed form
  sum(dist_r) = 2*N*sum(sq_r) - 2*||colsum(X_r)||^2. The joint kernel is
  exp(E) in a single matmul + exp. Symmetry of E halves the work via a
  block-cyclic cover: core k owns row-chunks {k, k+8} (chunk = 512 rows)
  and computes 17 [512 x 512] blocks — column offsets d=0..8 from row
  chunk k, d=0..7 from row chunk k+8 — counting every unordered
  off-diagonal chunk pair exactly once (weight 2) and diagonals once
  (weight 1). Per-block sums (fp32, one per PSUM partition) return to the
  host, which applies weights/signs and the final reduction in float64.

Mixed-precision contraction (K = 322 total):
  - rows 0..255  (the sqrt(g0)*X0 block of W): float8_e4m3 scaled by
    s=128, contracted in ONE DoubleRow matmul (2 fp8 rows per PE cell,
    K=256 in a single 128-partition pass)
  - rows 256..321 (sqrt(g1)*X1, ones, -c): bf16 with the s^2 scale
    folded into the lhs, one ordinary 66-partition pass
  Two matmul instructions per m-tile instead of three 512-col bf16
  passes. The Exp activation applies scale=1/s^2 and accumulates the
  per-row block sum into the acc column.

Per-core device program (SPMD — identical instructions, data differs):
  - lhs8 [2, 128, 2, 512] fp8 / lhsb [2, 66, 512] bf16: stationary rows
    for row-chunks k, k+8
  - rhs8 [16, 128, 2, 512] fp8 / rhsb [16, 66, 512] bf16: moving
    columns, chunk-major with chunk order rotated by k
  - 17 blocks x 4 m-tiles: DoubleRow fp8 matmul (start) + bf16 matmul
    (stop) into PSUM [128, 2048] (4 banks), one Exp activation with
    accum_out producing the per-partition block sum
  - out "acc" [128, 17] fp32
"""

import os

import ml_dtypes
import numpy as np

import concourse.bacc as bacc
import concourse.bass as bass
import concourse.mybir as mybir
import concourse.tile as tile
from concourse.bass_utils import run_bass_kernel_spmd

B = 4096
D0, D1 = 256, 64
N = 2 * B
CH = 512          # rows per chunk
NCHUNK = 16
NCORE = 8
KA = 128          # fp8 DoubleRow partitions (contraction rows 0..255 = X0)
KB = D1 + 2       # bf16 partitions (X1 rows + ones + -c) = 66
MT = 128          # m-tile rows
NMT = CH // MT    # m-tiles per row-chunk (4)
NBLK = 17         # blocks per core (9 from chunk k, 8 from chunk k+8)
NCOL = NBLK       # acc columns

F8 = mybir.dt.float8e4
BF16 = mybir.dt.bfloat16
NP8 = ml_dtypes.float8_e4m3
NPB = ml_dtypes.bfloat16
S = 128.0         # fp8 scale on the X0 block of W
ASCALE = 1.0 / (S * S)

_N_WARMUP = int(os.environ.get("JMMD_WARMUP", "7"))
# "act": Exp activation accumulates the block sum (accum_out + 284ns
# accumulator read per block on the Scalar queue).
# "dve": Exp writes bf16 to SBUF and the (otherwise idle) Vector engine
# reduces, trimming the Scalar queue to the exp stream itself.
_REDUCE = os.environ.get("JMMD_REDUCE", "act")

LAST_EXEC_NS = None
LAST_RESULTS = None

_CACHE: dict = {}


def _build():
    if "nc" in _CACHE:
        return _CACHE["nc"]
    nc = bacc.Bacc(
        "TRN2", target_bir_lowering=False, debug=False, enable_asserts=False
    )
    f32 = mybir.dt.float32
    lhs8_dram = nc.dram_tensor("lhs8", [2, KA, 2, CH], F8, kind="ExternalInput").ap()
    lhsb_dram = nc.dram_tensor("lhsb", [2, KB, CH], BF16, kind="ExternalInput").ap()
    rhs8_dram = nc.dram_tensor(
        "rhs8", [NCHUNK, KA, 2, CH], F8, kind="ExternalInput"
    ).ap()
    rhsb_dram = nc.dram_tensor(
        "rhsb", [NCHUNK, KB, CH], BF16, kind="ExternalInput"
    ).ap()
    acc_dram = nc.dram_tensor("acc", [MT, NCOL], f32, kind="ExternalOutput").ap()

    DR = mybir.MatmulPerfMode.DoubleRow

    with tile.TileContext(nc) as tc:
        with (
            tc.tile_pool(name="const", bufs=1) as const,
            tc.tile_pool(name="psum", bufs=2, space=bass.MemorySpace.PSUM) as psum,
        ):
            lhs_t = {}
            rhs_t = {}

            def load_lhs(g, eng):
                ta = const.tile([KA, 2, CH], F8, tag=f"lhs8_{g}")
                tb = const.tile([KB, CH], BF16, tag=f"lhsb_{g}")
                eng.dma_start(ta[:], lhs8_dram[g])
                eng.dma_start(tb[:], lhsb_dram[g])
                lhs_t[g] = (ta, tb)

            def load_rhs(ch, eng):
                ta = const.tile([KA, 2, CH], F8, tag=f"rhs8_{ch}")
                tb = const.tile([KB, CH], BF16, tag=f"rhsb_{ch}")
                eng.dma_start(ta[:], rhs8_dram[ch])
                eng.dma_start(tb[:], rhsb_dram[ch])
                rhs_t[ch] = (ta, tb)

            # warmup scratch memset goes FIRST on gpsimd — anything queued
            # behind bulk DMAs on that engine would stall the PE program.
            # 512-col bf16 warmup streams: short [128x128] matmuls never
            # trigger the HAM 8/8 un-throttle, and DoubleRow matmuls hold it
            # down — only a sustained stretch of full-width bf16 streams
            # flips the PE to full clock (measured: ~5us of 512-col bf16).
            scratch = None
            if _N_WARMUP:
                scratch = const.tile([MT, MT + CH], BF16, tag="warm_src")
                nc.vector.memset(scratch[:], 0.0)

            # block 0's operands race down both DMA engines in parallel;
            # lhs g=1 is not needed until block 9
            load_rhs(0, nc.gpsimd)
            load_lhs(0, nc.sync)
            for ch in (1, 3):
                load_rhs(ch, nc.sync)
            for ch in (2, 4):
                load_rhs(ch, nc.gpsimd)
            load_lhs(1, nc.sync)
            for ch in (5, 7, 9, 11):
                load_rhs(ch, nc.sync)
            for ch in (6, 8, 10, 12):
                load_rhs(ch, nc.gpsimd)
            for ch in (13, 15):
                load_rhs(ch, nc.sync)
            load_rhs(14, nc.gpsimd)

            acc_t = const.tile([MT, NCOL], f32, tag="acc")

            # HAM warmup: dummy matmuls while input DMAs stream, so real
            # matmuls start past the cold PE p-state.
            if _N_WARMUP:
                warm_ps = psum.tile([MT, NMT * CH], f32, tag="ps")
                for _ in range(_N_WARMUP):
                    nc.tensor.matmul(
                        warm_ps[:, :CH],
                        scratch[:, :MT],
                        scratch[:, MT:],
                        start=True,
                        stop=True,
                    )

            for g, nd in ((0, 9), (1, 8)):
                la, lb = lhs_t[g]
                for d in range(nd):
                    ch = d if g == 0 else 8 + d
                    col = d if g == 0 else 9 + d
                    ra, rb = rhs_t[ch]
                    ps = psum.tile([MT, NMT * CH], f32, tag="ps")
                    for m in range(NMT):
                        ms = slice(m * MT, (m + 1) * MT)
                        nc.tensor.matmul(
                            ps[:, m * CH:(m + 1) * CH],
                            la[:, :, ms],
                            ra[:],
                            start=True,
                            stop=False,
                            perf_mode=DR,
                        )
                        nc.tensor.matmul(
                            ps[:, m * CH:(m + 1) * CH],
                            lb[:, ms],
                            rb[:],
                            start=False,
                            stop=True,
                        )
                    if _REDUCE == "act":
                        nc.scalar.activation(
                            ps[:],
                            ps[:],
                            mybir.ActivationFunctionType.Exp,
                            scale=ASCALE,
                            accum_out=acc_t[:, col:col + 1],
                        )
                    else:
                        et = const.tile(
                            [MT, NMT * CH], BF16, tag=f"exp{col % 2}"
                        )
                        nc.scalar.activation(
                            et[:],
                            ps[:],
                            mybir.ActivationFunctionType.Exp,
                            scale=ASCALE,
                        )
                        nc.vector.reduce_sum(
                            acc_t[:, col:col + 1],
                            et[:],
                            axis=mybir.AxisListType.X,
                        )
            nc.sync.dma_start(acc_dram[:], acc_t[:])
    nc.compile()
    _CACHE["nc"] = nc
    return nc


def _pack_inputs(s0, s1, t0, t1):
    X0 = np.concatenate([s0, t0], axis=0).astype(np.float64)
    X1 = np.concatenate([s1, t1], axis=0).astype(np.float64)

    def gamma_of(X):
        sq = np.sum(X * X, axis=1)
        sdist = 2.0 * X.shape[0] * np.sum(sq) - 2.0 * np.sum(np.sum(X, axis=0) ** 2)
        return (X.shape[0] ** 2 - X.shape[0]) / sdist, sq

    g0, sq0 = gamma_of(X0)
    g1, sq1 = gamma_of(X1)
    c = g0 * sq0 + g1 * sq1
    W0 = np.sqrt(g0) * X0          # [N, 256]
    W1 = np.sqrt(g1) * X1          # [N, 64]

    # fp8 part: rows 0..255. lhs = 2sW0.T, rhs = sW0.T, DoubleRow packed
    # as [128, 2, N] with partition k sub i -> row i*128 + k.
    def pack_dr(mat):  # [256, N] f64 -> [128, 2, N] fp8
        m8 = np.clip(mat, -240.0, 240.0).astype(NP8)
        return np.stack([m8[:KA], m8[KA:]], axis=1)

    lhs8_full = pack_dr(2.0 * S * W0.T)
    rhs8_full = pack_dr(S * W0.T)

    # bf16 part: rows 256..321 = [X1-block; ones; -c], with the s^2 scale
    # folded into the lhs so both passes share one PSUM scale.
    S2 = S * S
    lhsb_full = np.empty((KB, N), dtype=np.float64)
    rhsb_full = np.empty((KB, N), dtype=np.float64)
    lhsb_full[:D1] = 2.0 * S2 * W1.T
    rhsb_full[:D1] = W1.T
    lhsb_full[D1] = S2           # * rhs -c_j
    rhsb_full[D1] = -c
    lhsb_full[D1 + 1] = -S2 * c  # * rhs ones
    rhsb_full[D1 + 1] = 1.0
    lhsb_full = lhsb_full.astype(NPB)
    rhsb_full = rhsb_full.astype(NPB)

    def chunk_major(full):
        # [..., N] -> [NCHUNK, ..., CH]
        lead = full.shape[:-1]
        arr = full.reshape(*lead, NCHUNK, CH)
        perm = (len(lead),) + tuple(range(len(lead))) + (len(lead) + 1,)
        return np.ascontiguousarray(arr.transpose(perm))

    r8 = chunk_major(rhs8_full)    # [16, 128, 2, 512]
    rb = chunk_major(rhsb_full)    # [16, 66, 512]
    l8 = chunk_major(lhs8_full)
    lb = chunk_major(lhsb_full)

    in_maps = []
    for k in range(NCORE):
        order = [(k + d) % NCHUNK for d in range(NCHUNK)]
        in_maps.append({
            "lhs8": np.stack([l8[k], l8[(k + 8) % NCHUNK]]),
            "lhsb": np.stack([lb[k], lb[(k + 8) % NCHUNK]]),
            "rhs8": np.ascontiguousarray(r8[order]),
            "rhsb": np.ascontiguousarray(rb[order]),
        })
    return in_maps


def _combine(results):
    sgn = lambda ch: 1.0 if ch < NCHUNK // 2 else -1.0
    total = 0.0
    for k in range(NCORE):
        acc = np.asarray(results[k]["acc"], dtype=np.float64)  # [128, 17]
        colsum = acc.sum(axis=0)
        for col in range(NCOL):
            if col < 9:
                d, row_chunk = col, k
            else:
                d, row_chunk = col - 9, (k + 8) % NCHUNK
            col_chunk = (row_chunk + d) % NCHUNK
            w = 1.0 if d == 0 else 2.0
            s = sgn(row_chunk) * sgn(col_chunk)
            total += w * s * colsum[col]
    return total / (B * B)


def kernel(s0, s1, t0, t1):
    global LAST_EXEC_NS, LAST_RESULTS
    nc = _build()
    in_maps = _pack_inputs(
        np.asarray(s0), np.asarray(s1), np.asarray(t0), np.asarray(t1)
    )
    trace = os.environ.get("JMMD_TRACE", "0") == "1"
    res = run_bass_kernel_spmd(nc, in_maps, core_ids=list(range(NCORE)), trace=trace)
    LAST_EXEC_NS = res.exec_time_ns
    LAST_RESULTS = res
    return np.float32(_combine(res.results))


# revision 10
# speedup vs baseline: 1.3104x; 1.3104x over previous
"""Joint Maximum Mean Discrepancy loss on 8 Trainium2 NeuronCores.

Math: for streams (s0,t0) and (s1,t1), the reference builds per-stream
Gaussian kernels K_r = exp(-gamma_r * dist_r) over feats_r = [src; tgt]
(N=8192 rows), takes their elementwise product, and returns
mean(s2s + t2t - 2*s2t) over the B x B blocks.

Device decomposition:
  exponent E_ij = 2*(W @ W.T)_ij - c_i - c_j
  with W = [sqrt(g0)*X0, sqrt(g1)*X1] (N x 320), c_i = g0*|X0_i|^2 +
  g1*|X1_i|^2, and gamma_r from the closed form
  sum(dist_r) = 2*N*sum(sq_r) - 2*||colsum(X_r)||^2. The joint kernel is
  exp(E) in a single matmul + exp. Symmetry of E halves the work via a
  block-cyclic cover: core k owns row-chunks {k, k+8} (chunk = 512 rows)
  and computes 17 [512 x 512] blocks — column offsets d=0..8 from row
  chunk k, d=0..7 from row chunk k+8 — counting every unordered
  off-diagonal chunk pair exactly once (weight 2) and diagonals once
  (weight 1). Per-block sums (fp32, one per PSUM partition) return to the
  host, which applies weights/signs and the final reduction in float64.

Mixed-precision contraction (K = 322 total):
  - rows 0..255  (the sqrt(g0)*X0 block of W): float8_e4m3 scaled by
    s=128, contracted in ONE DoubleRow matmul (2 fp8 rows per PE cell,
    K=256 in a single 128-partition pass)
  - rows 256..321 (sqrt(g1)*X1, ones, -c): bf16 with the s^2 scale
    folded into the lhs, one ordinary 66-partition pass
  Two matmul instructions per m-tile instead of three 512-col bf16
  passes. The Exp activation applies scale=1/s^2 and accumulates the
  per-row block sum into the acc column.

Per-core device program (SPMD — identical instructions, data differs):
  - lhs8 [2, 128, 2, 512] fp8 / lhsb [2, 66, 512] bf16: stationary rows
    for row-chunks k, k+8
  - rhs8 [16, 128, 2, 512] fp8 / rhsb [16, 66, 512] bf16: moving
    columns, chunk-major with chunk order rotated by k
  - 17 blocks x 4 m-tiles: DoubleRow fp8 matmul (start) + bf16 matmul
    (stop) into PSUM [128, 2048] (4 banks), one Exp activation with
    accum_out producing the per-partition block sum
  - out "acc" [128, 17] fp32
"""

import os

import ml_dtypes
import numpy as np

import concourse.bacc as bacc
import concourse.bass as bass
import concourse.mybir as mybir
import concourse.tile as tile
from concourse.bass_utils import run_bass_kernel_spmd

B = 4096
D0, D1 = 256, 64
N = 2 * B
CH = 512          # rows per chunk
NCHUNK = 16
NCORE = 8
KA = 128          # fp8 DoubleRow partitions (contraction rows 0..255 = X0)
KB = D1 + 2       # bf16 partitions (X1 rows + ones + -c) = 66
MT = 128          # m-tile rows
NMT = CH // MT    # m-tiles per row-chunk (4)
NBLK = 17         # blocks per core (9 from chunk k, 8 from chunk k+8)
NCOL = NBLK       # acc columns

F8 = mybir.dt.float8e4
BF16 = mybir.dt.bfloat16
NP8 = ml_dtypes.float8_e4m3
NPB = ml_dtypes.bfloat16
S = 128.0         # fp8 scale on the X0 block of W
ASCALE = 1.0 / (S * S)

_N_WARMUP = int(os.environ.get("JMMD_WARMUP", "14"))
# "act": Exp activation accumulates the block sum (accum_out + 284ns
# accumulator read per block on the Scalar queue).
# "dve": Exp writes bf16 to SBUF and the (otherwise idle) Vector engine
# reduces, trimming the Scalar queue to the exp stream itself.
_REDUCE = os.environ.get("JMMD_REDUCE", "act")

LAST_EXEC_NS = None
LAST_RESULTS = None

_CACHE: dict = {}


def _build():
    if "nc" in _CACHE:
        return _CACHE["nc"]
    nc = bacc.Bacc(
        "TRN2", target_bir_lowering=False, debug=False, enable_asserts=False
    )
    f32 = mybir.dt.float32
    lhs8_dram = nc.dram_tensor("lhs8", [2, KA, 2, CH], F8, kind="ExternalInput").ap()
    lhsb_dram = nc.dram_tensor("lhsb", [2, KB, CH], BF16, kind="ExternalInput").ap()
    rhs8_dram = nc.dram_tensor(
        "rhs8", [NCHUNK, KA, 2, CH], F8, kind="ExternalInput"
    ).ap()
    rhsb_dram = nc.dram_tensor(
        "rhsb", [NCHUNK, KB, CH], BF16, kind="ExternalInput"
    ).ap()
    acc_dram = nc.dram_tensor("acc", [MT, NCOL], f32, kind="ExternalOutput").ap()

    DR = mybir.MatmulPerfMode.DoubleRow

    with tile.TileContext(nc) as tc:
        with (
            tc.tile_pool(name="const", bufs=1) as const,
            tc.tile_pool(name="psum", bufs=2, space=bass.MemorySpace.PSUM) as psum,
        ):
            lhs_t = {}
            rhs_t = {}

            def load_lhs(g, eng):
                ta = const.tile([KA, 2, CH], F8, tag=f"lhs8_{g}")
                tb = const.tile([KB, CH], BF16, tag=f"lhsb_{g}")
                eng.dma_start(ta[:], lhs8_dram[g])
                eng.dma_start(tb[:], lhsb_dram[g])
                lhs_t[g] = (ta, tb)

            def load_rhs(ch, eng):
                ta = const.tile([KA, 2, CH], F8, tag=f"rhs8_{ch}")
                tb = const.tile([KB, CH], BF16, tag=f"rhsb_{ch}")
                eng.dma_start(ta[:], rhs8_dram[ch])
                eng.dma_start(tb[:], rhsb_dram[ch])
                rhs_t[ch] = (ta, tb)

            # 512-col bf16 warmup streams: short [128x128] matmuls never
            # trigger the HAM 8/8 un-throttle, and DoubleRow matmuls hold it
            # down — only a sustained stretch of full-width bf16 streams
            # flips the PE to full clock (measured: ~5us of 512-col bf16).
            # The memset goes FIRST on gpsimd — anything queued behind the
            # bulk DMAs on that engine would stall the PE program.
            scratch = None
            if _N_WARMUP:
                scratch = const.tile([MT, MT + CH], BF16, tag="warm_src")
                nc.gpsimd.memset(scratch[:], 0.0)

            # block 0's operands race down both DMA engines in parallel;
            # lhs g=1 is not needed until block 9
            load_rhs(0, nc.gpsimd)
            load_lhs(0, nc.sync)
            for ch in (1, 3):
                load_rhs(ch, nc.sync)
            for ch in (2, 4):
                load_rhs(ch, nc.gpsimd)
            load_lhs(1, nc.sync)
            for ch in (5, 7, 9, 11):
                load_rhs(ch, nc.sync)
            for ch in (6, 8, 10, 12):
                load_rhs(ch, nc.gpsimd)
            for ch in (13, 15):
                load_rhs(ch, nc.sync)
            load_rhs(14, nc.gpsimd)

            acc_t = const.tile([MT, NCOL], f32, tag="acc")

            # HAM warmup: dummy matmuls while input DMAs stream, so real
            # matmuls start past the cold PE p-state.
            if _N_WARMUP:
                warm_ps = psum.tile([MT, NMT * CH], f32, tag="ps")
                for _ in range(_N_WARMUP):
                    nc.tensor.matmul(
                        warm_ps[:, :CH],
                        scratch[:, :MT],
                        scratch[:, MT:],
                        start=True,
                        stop=True,
                    )

            for g, nd in ((0, 9), (1, 8)):
                la, lb = lhs_t[g]
                for d in range(nd):
                    ch = d if g == 0 else 8 + d
                    col = d if g == 0 else 9 + d
                    ra, rb = rhs_t[ch]
                    ps = psum.tile([MT, NMT * CH], f32, tag="ps")
                    for m in range(NMT):
                        ms = slice(m * MT, (m + 1) * MT)
                        nc.tensor.matmul(
                            ps[:, m * CH:(m + 1) * CH],
                            la[:, :, ms],
                            ra[:],
                            start=True,
                            stop=False,
                            perf_mode=DR,
                        )
                        nc.tensor.matmul(
                            ps[:, m * CH:(m + 1) * CH],
                            lb[:, ms],
                            rb[:],
                            start=False,
                            stop=True,
                        )
                    if _REDUCE == "act":
                        nc.scalar.activation(
                            ps[:],
                            ps[:],
                            mybir.ActivationFunctionType.Exp,
                            scale=ASCALE,
                            accum_out=acc_t[:, col:col + 1],
                        )
                    else:
                        et = const.tile(
                            [MT, NMT * CH], BF16, tag=f"exp{col % 2}"
                        )
                        nc.scalar.activation(
                            et[:],
                            ps[:],
                            mybir.ActivationFunctionType.Exp,
                            scale=ASCALE,
                        )
                        nc.vector.reduce_sum(
                            acc_t[:, col:col + 1],
                            et[:],
                            axis=mybir.AxisListType.X,
                        )
            nc.sync.dma_start(acc_dram[:], acc_t[:])
    nc.compile()
    _CACHE["nc"] = nc
    return nc


def _pack_inputs(s0, s1, t0, t1):
    X0 = np.concatenate([s0, t0], axis=0).astype(np.float64)
    X1 = np.concatenate([s1, t1], axis=0).astype(np.float64)

    def gamma_of(X):
        sq = np.sum(X * X, axis=1)
        sdist = 2.0 * X.shape[0] * np.sum(sq) - 2.0 * np.sum(np.sum(X, axis=0) ** 2)
        return (X.shape[0] ** 2 - X.shape[0]) / sdist, sq

    g0, sq0 = gamma_of(X0)
    g1, sq1 = gamma_of(X1)
    c = g0 * sq0 + g1 * sq1
    W0 = np.sqrt(g0) * X0          # [N, 256]
    W1 = np.sqrt(g1) * X1          # [N, 64]

    # fp8 part: rows 0..255. lhs = 2sW0.T, rhs = sW0.T, DoubleRow packed
    # as [128, 2, N] with partition k sub i -> row i*128 + k.
    def pack_dr(mat):  # [256, N] f64 -> [128, 2, N] fp8
        m8 = np.clip(mat, -240.0, 240.0).astype(NP8)
        return np.stack([m8[:KA], m8[KA:]], axis=1)

    lhs8_full = pack_dr(2.0 * S * W0.T)
    rhs8_full = pack_dr(S * W0.T)

    # bf16 part: rows 256..321 = [X1-block; ones; -c], with the s^2 scale
    # folded into the lhs so both passes share one PSUM scale.
    S2 = S * S
    lhsb_full = np.empty((KB, N), dtype=np.float64)
    rhsb_full = np.empty((KB, N), dtype=np.float64)
    lhsb_full[:D1] = 2.0 * S2 * W1.T
    rhsb_full[:D1] = W1.T
    lhsb_full[D1] = S2           # * rhs -c_j
    rhsb_full[D1] = -c
    lhsb_full[D1 + 1] = -S2 * c  # * rhs ones
    rhsb_full[D1 + 1] = 1.0
    lhsb_full = lhsb_full.astype(NPB)
    rhsb_full = rhsb_full.astype(NPB)

    def chunk_major(full):
        # [..., N] -> [NCHUNK, ..., CH]
        lead = full.shape[:-1]
        arr = full.reshape(*lead, NCHUNK, CH)
        perm = (len(lead),) + tuple(range(len(lead))) + (len(lead) + 1,)
        return np.ascontiguousarray(arr.transpose(perm))

    r8 = chunk_major(rhs8_full)    # [16, 128, 2, 512]
    rb = chunk_major(rhsb_full)    # [16, 66, 512]
    l8 = chunk_major(lhs8_full)
    lb = chunk_major(lhsb_full)

    in_maps = []
    for k in range(NCORE):
        order = [(k + d) % NCHUNK for d in range(NCHUNK)]
        in_maps.append({
            "lhs8": np.stack([l8[k], l8[(k + 8) % NCHUNK]]),
            "lhsb": np.stack([lb[k], lb[(k + 8) % NCHUNK]]),
            "rhs8": np.ascontiguousarray(r8[order]),
            "rhsb": np.ascontiguousarray(rb[order]),
        })
    return in_maps


def _combine(results):
    sgn = lambda ch: 1.0 if ch < NCHUNK // 2 else -1.0
    total = 0.0
    for k in range(NCORE):
        acc = np.asarray(results[k]["acc"], dtype=np.float64)  # [128, 17]
        colsum = acc.sum(axis=0)
        for col in range(NCOL):
            if col < 9:
                d, row_chunk = col, k
            else:
                d, row_chunk = col - 9, (k + 8) % NCHUNK
            col_chunk = (row_chunk + d) % NCHUNK
            w = 1.0 if d == 0 else 2.0
            s = sgn(row_chunk) * sgn(col_chunk)
            total += w * s * colsum[col]
    return total / (B * B)


def kernel(s0, s1, t0, t1):
    global LAST_EXEC_NS, LAST_RESULTS
    nc = _build()
    in_maps = _pack_inputs(
        np.asarray(s0), np.asarray(s1), np.asarray(t0), np.asarray(t1)
    )
    trace = os.environ.get("JMMD_TRACE", "0") == "1"
    res = run_bass_kernel_spmd(nc, in_maps, core_ids=list(range(NCORE)), trace=trace)
    LAST_EXEC_NS = res.exec_time_ns
    LAST_RESULTS = res
    return np.float32(_combine(res.results))


# revision 11
# speedup vs baseline: 1.3721x; 1.0471x over previous
"""Joint Maximum Mean Discrepancy loss on 8 Trainium2 NeuronCores.

Math: for streams (s0,t0) and (s1,t1), the reference builds per-stream
Gaussian kernels K_r = exp(-gamma_r * dist_r) over feats_r = [src; tgt]
(N=8192 rows), takes their elementwise product, and returns
mean(s2s + t2t - 2*s2t) over the B x B blocks.

Device decomposition:
  exponent E_ij = 2*(W @ W.T)_ij - c_i - c_j
  with W = [sqrt(g0)*X0, sqrt(g1)*X1] (N x 320), c_i = g0*|X0_i|^2 +
  g1*|X1_i|^2, and gamma_r from the closed form
  sum(dist_r) = 2*N*sum(sq_r) - 2*||colsum(X_r)||^2. The joint kernel is
  exp(E) in a single matmul + exp. Symmetry of E halves the work via a
  block-cyclic cover: core k owns row-chunks {k, k+8} (chunk = 512 rows)
  and computes 17 [512 x 512] blocks — column offsets d=0..8 from row
  chunk k, d=0..7 from row chunk k+8 — counting every unordered
  off-diagonal chunk pair exactly once (weight 2) and diagonals once
  (weight 1). Per-block sums (fp32, one per PSUM partition) return to the
  host, which applies weights/signs and the final reduction in float64.

Mixed-precision contraction (K = 322 total):
  - rows 0..255  (the sqrt(g0)*X0 block of W): float8_e4m3 scaled by
    s=128, contracted in ONE DoubleRow matmul (2 fp8 rows per PE cell,
    K=256 in a single 128-partition pass)
  - rows 256..321 (sqrt(g1)*X1, ones, -c): bf16 with the s^2 scale
    folded into the lhs, one ordinary 66-partition pass
  Two matmul instructions per m-tile instead of three 512-col bf16
  passes. The Exp activation applies scale=1/s^2 and accumulates the
  per-row block sum into the acc column.

Per-core device program (SPMD — identical instructions, data differs):
  - lhs8 [2, 128, 2, 512] fp8 / lhsb [2, 66, 512] bf16: stationary rows
    for row-chunks k, k+8
  - rhs8 [16, 128, 2, 512] fp8 / rhsb [16, 66, 512] bf16: moving
    columns, chunk-major with chunk order rotated by k
  - 17 blocks x 4 m-tiles: DoubleRow fp8 matmul (start) + bf16 matmul
    (stop) into PSUM [128, 2048] (4 banks), one Exp activation with
    accum_out producing the per-partition block sum
  - out "acc" [128, 17] fp32
"""

import os

import ml_dtypes
import numpy as np

import concourse.bacc as bacc
import concourse.bass as bass
import concourse.mybir as mybir
import concourse.tile as tile
from concourse.bass_utils import run_bass_kernel_spmd

B = 4096
D0, D1 = 256, 64
N = 2 * B
CH = 512          # rows per chunk
NCHUNK = 16
NCORE = 8
KA = 128          # fp8 DoubleRow partitions (contraction rows 0..255 = X0)
KB = D1 + 2       # bf16 partitions (X1 rows + ones + -c) = 66
MT = 128          # m-tile rows
NMT = CH // MT    # m-tiles per row-chunk (4)
NBLK = 17         # blocks per core (9 from chunk k, 8 from chunk k+8)
NCOL = NBLK       # acc columns

F8 = mybir.dt.float8e4
BF16 = mybir.dt.bfloat16
NP8 = ml_dtypes.float8_e4m3
NPB = ml_dtypes.bfloat16
S = 128.0         # fp8 scale on the X0 block of W
ASCALE = 1.0 / (S * S)

_N_WARMUP = int(os.environ.get("JMMD_WARMUP", "14"))
# "act": Exp activation accumulates the block sum (accum_out + 284ns
# accumulator read per block on the Scalar queue).
# "dve": Exp writes bf16 to SBUF and the (otherwise idle) Vector engine
# reduces, trimming the Scalar queue to the exp stream itself.
_REDUCE = os.environ.get("JMMD_REDUCE", "act")

LAST_EXEC_NS = None
LAST_RESULTS = None

_CACHE: dict = {}


def _build():
    if "nc" in _CACHE:
        return _CACHE["nc"]
    nc = bacc.Bacc(
        "TRN2", target_bir_lowering=False, debug=False, enable_asserts=False
    )
    f32 = mybir.dt.float32
    lhs8_dram = nc.dram_tensor("lhs8", [2, KA, 2, CH], F8, kind="ExternalInput").ap()
    lhsb_dram = nc.dram_tensor("lhsb", [2, KB, CH], BF16, kind="ExternalInput").ap()
    rhs8_dram = nc.dram_tensor(
        "rhs8", [NCHUNK, KA, 2, CH], F8, kind="ExternalInput"
    ).ap()
    rhsb_dram = nc.dram_tensor(
        "rhsb", [NCHUNK, KB, CH], BF16, kind="ExternalInput"
    ).ap()
    acc_dram = nc.dram_tensor("acc", [MT, NCOL], f32, kind="ExternalOutput").ap()

    DR = mybir.MatmulPerfMode.DoubleRow

    with tile.TileContext(nc) as tc:
        with (
            tc.tile_pool(name="const", bufs=1) as const,
            tc.tile_pool(name="psum", bufs=2, space=bass.MemorySpace.PSUM) as psum,
        ):
            lhs_t = {}
            rhs_t = {}

            def load_lhs(g, eng):
                ta = const.tile([KA, 2, CH], F8, tag=f"lhs8_{g}")
                tb = const.tile([KB, CH], BF16, tag=f"lhsb_{g}")
                eng.dma_start(ta[:], lhs8_dram[g])
                eng.dma_start(tb[:], lhsb_dram[g])
                lhs_t[g] = (ta, tb)

            def load_rhs(ch, eng):
                ta = const.tile([KA, 2, CH], F8, tag=f"rhs8_{ch}")
                tb = const.tile([KB, CH], BF16, tag=f"rhsb_{ch}")
                eng.dma_start(ta[:], rhs8_dram[ch])
                eng.dma_start(tb[:], rhsb_dram[ch])
                rhs_t[ch] = (ta, tb)

            # 512-col bf16 warmup streams: short [128x128] matmuls never
            # trigger the HAM 8/8 un-throttle, and DoubleRow matmuls hold it
            # down — only a sustained stretch of full-width bf16 streams
            # flips the PE to full clock (measured: ~5us of 512-col bf16).
            # The scratch zeroing runs on the Scalar engine so the gpsimd
            # queue goes straight to chunk-0's DMA and warmup starts the
            # moment the engines release.
            scratch = None
            if _N_WARMUP:
                scratch = const.tile([MT, MT + CH], BF16, tag="warm_src")
                nc.scalar.memzero(scratch[:])

            # block 0's operands race down both DMA engines in parallel;
            # lhs g=1 is not needed until block 9
            load_rhs(0, nc.gpsimd)
            load_lhs(0, nc.sync)
            for ch in (1, 3):
                load_rhs(ch, nc.sync)
            for ch in (2, 4):
                load_rhs(ch, nc.gpsimd)
            load_lhs(1, nc.sync)
            for ch in (5, 7, 9, 11):
                load_rhs(ch, nc.sync)
            for ch in (6, 8, 10, 12):
                load_rhs(ch, nc.gpsimd)
            for ch in (13, 15):
                load_rhs(ch, nc.sync)
            load_rhs(14, nc.gpsimd)

            acc_t = const.tile([MT, NCOL], f32, tag="acc")

            # HAM warmup: dummy matmuls while input DMAs stream, so real
            # matmuls start past the cold PE p-state.
            if _N_WARMUP:
                warm_ps = psum.tile([MT, NMT * CH], f32, tag="ps")
                for _ in range(_N_WARMUP):
                    nc.tensor.matmul(
                        warm_ps[:, :CH],
                        scratch[:, :MT],
                        scratch[:, MT:],
                        start=True,
                        stop=True,
                    )

            for g, nd in ((0, 9), (1, 8)):
                la, lb = lhs_t[g]
                for d in range(nd):
                    ch = d if g == 0 else 8 + d
                    col = d if g == 0 else 9 + d
                    ra, rb = rhs_t[ch]
                    ps = psum.tile([MT, NMT * CH], f32, tag="ps")
                    for m in range(NMT):
                        ms = slice(m * MT, (m + 1) * MT)
                        nc.tensor.matmul(
                            ps[:, m * CH:(m + 1) * CH],
                            la[:, :, ms],
                            ra[:],
                            start=True,
                            stop=False,
                            perf_mode=DR,
                        )
                        nc.tensor.matmul(
                            ps[:, m * CH:(m + 1) * CH],
                            lb[:, ms],
                            rb[:],
                            start=False,
                            stop=True,
                        )
                    if _REDUCE == "act":
                        nc.scalar.activation(
                            ps[:],
                            ps[:],
                            mybir.ActivationFunctionType.Exp,
                            scale=ASCALE,
                            accum_out=acc_t[:, col:col + 1],
                        )
                    else:
                        et = const.tile(
                            [MT, NMT * CH], BF16, tag=f"exp{col % 2}"
                        )
                        nc.scalar.activation(
                            et[:],
                            ps[:],
                            mybir.ActivationFunctionType.Exp,
                            scale=ASCALE,
                        )
                        nc.vector.reduce_sum(
                            acc_t[:, col:col + 1],
                            et[:],
                            axis=mybir.AxisListType.X,
                        )
            nc.sync.dma_start(acc_dram[:], acc_t[:])
    nc.compile()
    _CACHE["nc"] = nc
    return nc


def _pack_inputs(s0, s1, t0, t1):
    X0 = np.concatenate([s0, t0], axis=0).astype(np.float64)
    X1 = np.concatenate([s1, t1], axis=0).astype(np.float64)

    def gamma_of(X):
        sq = np.sum(X * X, axis=1)
        sdist = 2.0 * X.shape[0] * np.sum(sq) - 2.0 * np.sum(np.sum(X, axis=0) ** 2)
        return (X.shape[0] ** 2 - X.shape[0]) / sdist, sq

    g0, sq0 = gamma_of(X0)
    g1, sq1 = gamma_of(X1)
    c = g0 * sq0 + g1 * sq1
    W0 = np.sqrt(g0) * X0          # [N, 256]
    W1 = np.sqrt(g1) * X1          # [N, 64]

    # fp8 part: rows 0..255. lhs = 2sW0.T, rhs = sW0.T, DoubleRow packed
    # as [128, 2, N] with partition k sub i -> row i*128 + k.
    def pack_dr(mat):  # [256, N] f64 -> [128, 2, N] fp8
        m8 = np.clip(mat, -240.0, 240.0).astype(NP8)
        return np.stack([m8[:KA], m8[KA:]], axis=1)

    lhs8_full = pack_dr(2.0 * S * W0.T)
    rhs8_full = pack_dr(S * W0.T)

    # bf16 part: rows 256..321 = [X1-block; ones; -c], with the s^2 scale
    # folded into the lhs so both passes share one PSUM scale.
    S2 = S * S
    lhsb_full = np.empty((KB, N), dtype=np.float64)
    rhsb_full = np.empty((KB, N), dtype=np.float64)
    lhsb_full[:D1] = 2.0 * S2 * W1.T
    rhsb_full[:D1] = W1.T
    lhsb_full[D1] = S2           # * rhs -c_j
    rhsb_full[D1] = -c
    lhsb_full[D1 + 1] = -S2 * c  # * rhs ones
    rhsb_full[D1 + 1] = 1.0
    lhsb_full = lhsb_full.astype(NPB)
    rhsb_full = rhsb_full.astype(NPB)

    def chunk_major(full):
        # [..., N] -> [NCHUNK, ..., CH]
        lead = full.shape[:-1]
        arr = full.reshape(*lead, NCHUNK, CH)
        perm = (len(lead),) + tuple(range(len(lead))) + (len(lead) + 1,)
        return np.ascontiguousarray(arr.transpose(perm))

    r8 = chunk_major(rhs8_full)    # [16, 128, 2, 512]
    rb = chunk_major(rhsb_full)    # [16, 66, 512]
    l8 = chunk_major(lhs8_full)
    lb = chunk_major(lhsb_full)

    in_maps = []
    for k in range(NCORE):
        order = [(k + d) % NCHUNK for d in range(NCHUNK)]
        in_maps.append({
            "lhs8": np.stack([l8[k], l8[(k + 8) % NCHUNK]]),
            "lhsb": np.stack([lb[k], lb[(k + 8) % NCHUNK]]),
            "rhs8": np.ascontiguousarray(r8[order]),
            "rhsb": np.ascontiguousarray(rb[order]),
        })
    return in_maps


def _combine(results):
    sgn = lambda ch: 1.0 if ch < NCHUNK // 2 else -1.0
    total = 0.0
    for k in range(NCORE):
        acc = np.asarray(results[k]["acc"], dtype=np.float64)  # [128, 17]
        colsum = acc.sum(axis=0)
        for col in range(NCOL):
            if col < 9:
                d, row_chunk = col, k
            else:
                d, row_chunk = col - 9, (k + 8) % NCHUNK
            col_chunk = (row_chunk + d) % NCHUNK
            w = 1.0 if d == 0 else 2.0
            s = sgn(row_chunk) * sgn(col_chunk)
            total += w * s * colsum[col]
    return total / (B * B)


def kernel(s0, s1, t0, t1):
    global LAST_EXEC_NS, LAST_RESULTS
    nc = _build()
    in_maps = _pack_inputs(
        np.asarray(s0), np.asarray(s1), np.asarray(t0), np.asarray(t1)
    )
    trace = os.environ.get("JMMD_TRACE", "0") == "1"
    res = run_bass_kernel_spmd(nc, in_maps, core_ids=list(range(NCORE)), trace=trace)
    LAST_EXEC_NS = res.exec_time_ns
    LAST_RESULTS = res
    return np.float32(_combine(res.results))


# revision 12
# speedup vs baseline: 1.4078x; 1.0260x over previous
"""Joint Maximum Mean Discrepancy loss on 8 Trainium2 NeuronCores.

Math: for streams (s0,t0) and (s1,t1), the reference builds per-stream
Gaussian kernels K_r = exp(-gamma_r * dist_r) over feats_r = [src; tgt]
(N=8192 rows), takes their elementwise product, and returns
mean(s2s + t2t - 2*s2t) over the B x B blocks.

Device decomposition:
  exponent E_ij = 2*(W @ W.T)_ij - c_i - c_j
  with W = [sqrt(g0)*X0, sqrt(g1)*X1] (N x 320), c_i = g0*|X0_i|^2 +
  g1*|X1_i|^2, and gamma_r from the closed form
  sum(dist_r) = 2*N*sum(sq_r) - 2*||colsum(X_r)||^2. The joint kernel is
  exp(E) in a single matmul + exp. Symmetry of E halves the work via a
  block-cyclic cover: core k owns row-chunks {k, k+8} (chunk = 512 rows)
  and computes 17 [512 x 512] blocks — column offsets d=0..8 from row
  chunk k, d=0..7 from row chunk k+8 — counting every unordered
  off-diagonal chunk pair exactly once (weight 2) and diagonals once
  (weight 1). Per-block sums (fp32, one per PSUM partition) return to the
  host, which applies weights/signs and the final reduction in float64.

Mixed-precision contraction (K = 322 total):
  - rows 0..255  (the sqrt(g0)*X0 block of W): float8_e4m3 scaled by
    s=128, contracted in ONE DoubleRow matmul (2 fp8 rows per PE cell,
    K=256 in a single 128-partition pass)
  - rows 256..321 (sqrt(g1)*X1, ones, -c): bf16 with the s^2 scale
    folded into the lhs, one ordinary 66-partition pass
  Two matmul instructions per m-tile instead of three 512-col bf16
  passes. The Exp activation applies scale=1/s^2 and accumulates the
  per-row block sum into the acc column.

Per-core device program (SPMD — identical instructions, data differs):
  - lhs8 [2, 128, 2, 512] fp8 / lhsb [2, 66, 512] bf16: stationary rows
    for row-chunks k, k+8
  - rhs8 [16, 128, 2, 512] fp8 / rhsb [16, 66, 512] bf16: moving
    columns, chunk-major with chunk order rotated by k
  - 17 blocks x 4 m-tiles: DoubleRow fp8 matmul (start) + bf16 matmul
    (stop) into PSUM [128, 2048] (4 banks), one Exp activation with
    accum_out producing the per-partition block sum
  - out "acc" [128, 17] fp32
"""

import os

import ml_dtypes
import numpy as np

import concourse.bacc as bacc
import concourse.bass as bass
import concourse.mybir as mybir
import concourse.tile as tile
from concourse.bass_utils import run_bass_kernel_spmd

B = 4096
D0, D1 = 256, 64
N = 2 * B
CH = 512          # rows per chunk
NCHUNK = 16
NCORE = 8
KA = 128          # fp8 DoubleRow partitions (contraction rows 0..255 = X0)
KB = D1 + 2       # bf16 partitions (X1 rows + ones + -c) = 66
MT = 128          # m-tile rows
NMT = CH // MT    # m-tiles per row-chunk (4)
NBLK = 17         # blocks per core (9 from chunk k, 8 from chunk k+8)
NCOL = NBLK       # acc columns

F8 = mybir.dt.float8e4
BF16 = mybir.dt.bfloat16
NP8 = ml_dtypes.float8_e4m3
NPB = ml_dtypes.bfloat16
S = 128.0         # fp8 scale on the X0 block of W
ASCALE = 1.0 / (S * S)

_N_WARMUP = int(os.environ.get("JMMD_WARMUP", "14"))
# "act": Exp activation accumulates the block sum (accum_out + 284ns
# accumulator read per block on the Scalar queue).
# "dve": Exp writes bf16 to SBUF and the (otherwise idle) Vector engine
# reduces, trimming the Scalar queue to the exp stream itself.
_REDUCE = os.environ.get("JMMD_REDUCE", "act")

LAST_EXEC_NS = None
LAST_RESULTS = None

_CACHE: dict = {}


def _build():
    if "nc" in _CACHE:
        return _CACHE["nc"]
    nc = bacc.Bacc(
        "TRN2", target_bir_lowering=False, debug=False, enable_asserts=False
    )
    f32 = mybir.dt.float32
    lhs8_dram = nc.dram_tensor("lhs8", [2, KA, 2, CH], F8, kind="ExternalInput").ap()
    lhsb_dram = nc.dram_tensor("lhsb", [2, KB, CH], BF16, kind="ExternalInput").ap()
    rhs8_dram = nc.dram_tensor(
        "rhs8", [NCHUNK, KA, 2, CH], F8, kind="ExternalInput"
    ).ap()
    rhsb_dram = nc.dram_tensor(
        "rhsb", [NCHUNK, KB, CH], BF16, kind="ExternalInput"
    ).ap()
    acc_dram = nc.dram_tensor("acc", [MT, NCOL], f32, kind="ExternalOutput").ap()

    DR = mybir.MatmulPerfMode.DoubleRow

    with tile.TileContext(nc) as tc:
        with (
            tc.tile_pool(name="const", bufs=1) as const,
            tc.tile_pool(name="psum", bufs=2, space=bass.MemorySpace.PSUM) as psum,
        ):
            lhs_t = {}
            rhs_t = {}

            def load_lhs(g, eng):
                ta = const.tile([KA, 2, CH], F8, tag=f"lhs8_{g}")
                tb = const.tile([KB, CH], BF16, tag=f"lhsb_{g}")
                eng.dma_start(ta[:], lhs8_dram[g])
                eng.dma_start(tb[:], lhsb_dram[g])
                lhs_t[g] = (ta, tb)

            def load_rhs(ch, eng):
                ta = const.tile([KA, 2, CH], F8, tag=f"rhs8_{ch}")
                tb = const.tile([KB, CH], BF16, tag=f"rhsb_{ch}")
                eng.dma_start(ta[:], rhs8_dram[ch])
                eng.dma_start(tb[:], rhsb_dram[ch])
                rhs_t[ch] = (ta, tb)

            # 512-col bf16 warmup streams: short [128x128] matmuls never
            # trigger the HAM 8/8 un-throttle, and DoubleRow matmuls hold it
            # down — only a sustained stretch of full-width bf16 streams
            # flips the PE to full clock (measured: ~5us of 512-col bf16).
            # The memset goes FIRST on gpsimd — anything queued behind the
            # bulk DMAs on that engine would stall the PE program.
            scratch = None
            if _N_WARMUP:
                scratch = const.tile([MT, MT + CH], BF16, tag="warm_src")
                nc.gpsimd.memset(scratch[:], 0.0)

            # block 0's operands race down both DMA engines in parallel;
            # lhs g=1 is not needed until block 9
            load_rhs(0, nc.gpsimd)
            load_lhs(0, nc.sync)
            for ch in (1, 3):
                load_rhs(ch, nc.sync)
            for ch in (2, 4):
                load_rhs(ch, nc.gpsimd)
            load_lhs(1, nc.sync)
            for ch in (5, 7, 9, 11):
                load_rhs(ch, nc.sync)
            for ch in (6, 8, 10, 12):
                load_rhs(ch, nc.gpsimd)
            for ch in (13, 15):
                load_rhs(ch, nc.sync)
            load_rhs(14, nc.gpsimd)

            acc_t = const.tile([MT, NCOL], f32, tag="acc")

            # HAM warmup: dummy matmuls while input DMAs stream, so real
            # matmuls start past the cold PE p-state.
            if _N_WARMUP:
                warm_ps = psum.tile([MT, NMT * CH], f32, tag="ps")
                for _ in range(_N_WARMUP):
                    nc.tensor.matmul(
                        warm_ps[:, :CH],
                        scratch[:, :MT],
                        scratch[:, MT:],
                        start=True,
                        stop=True,
                    )

            for g, nd in ((0, 9), (1, 8)):
                la, lb = lhs_t[g]
                for d in range(nd):
                    ch = d if g == 0 else 8 + d
                    col = d if g == 0 else 9 + d
                    ra, rb = rhs_t[ch]
                    ps = psum.tile([MT, NMT * CH], f32, tag="ps")
                    for m in range(NMT):
                        ms = slice(m * MT, (m + 1) * MT)
                        nc.tensor.matmul(
                            ps[:, m * CH:(m + 1) * CH],
                            la[:, :, ms],
                            ra[:],
                            start=True,
                            stop=False,
                            perf_mode=DR,
                        )
                        nc.tensor.matmul(
                            ps[:, m * CH:(m + 1) * CH],
                            lb[:, ms],
                            rb[:],
                            start=False,
                            stop=True,
                        )
                    if _REDUCE == "act":
                        nc.scalar.activation(
                            ps[:],
                            ps[:],
                            mybir.ActivationFunctionType.Exp,
                            scale=ASCALE,
                            accum_out=acc_t[:, col:col + 1],
                        )
                    else:
                        et = const.tile(
                            [MT, NMT * CH], BF16, tag=f"exp{col % 2}"
                        )
                        nc.scalar.activation(
                            et[:],
                            ps[:],
                            mybir.ActivationFunctionType.Exp,
                            scale=ASCALE,
                        )
                        nc.vector.reduce_sum(
                            acc_t[:, col:col + 1],
                            et[:],
                            axis=mybir.AxisListType.X,
                        )
            nc.sync.dma_start(acc_dram[:], acc_t[:])
    nc.compile()
    _CACHE["nc"] = nc
    return nc


def _pack_inputs(s0, s1, t0, t1):
    X0 = np.concatenate([s0, t0], axis=0).astype(np.float64)
    X1 = np.concatenate([s1, t1], axis=0).astype(np.float64)

    def gamma_of(X):
        sq = np.sum(X * X, axis=1)
        sdist = 2.0 * X.shape[0] * np.sum(sq) - 2.0 * np.sum(np.sum(X, axis=0) ** 2)
        return (X.shape[0] ** 2 - X.shape[0]) / sdist, sq

    g0, sq0 = gamma_of(X0)
    g1, sq1 = gamma_of(X1)
    c = g0 * sq0 + g1 * sq1
    W0 = np.sqrt(g0) * X0          # [N, 256]
    W1 = np.sqrt(g1) * X1          # [N, 64]

    # fp8 part: rows 0..255. lhs = 2sW0.T, rhs = sW0.T, DoubleRow packed
    # as [128, 2, N] with partition k sub i -> row i*128 + k.
    def pack_dr(mat):  # [256, N] f64 -> [128, 2, N] fp8
        m8 = np.clip(mat, -240.0, 240.0).astype(NP8)
        return np.stack([m8[:KA], m8[KA:]], axis=1)

    lhs8_full = pack_dr(2.0 * S * W0.T)
    rhs8_full = pack_dr(S * W0.T)

    # bf16 part: rows 256..321 = [X1-block; ones; -c], with the s^2 scale
    # folded into the lhs so both passes share one PSUM scale.
    S2 = S * S
    lhsb_full = np.empty((KB, N), dtype=np.float64)
    rhsb_full = np.empty((KB, N), dtype=np.float64)
    lhsb_full[:D1] = 2.0 * S2 * W1.T
    rhsb_full[:D1] = W1.T
    lhsb_full[D1] = S2           # * rhs -c_j
    rhsb_full[D1] = -c
    lhsb_full[D1 + 1] = -S2 * c  # * rhs ones
    rhsb_full[D1 + 1] = 1.0
    lhsb_full = lhsb_full.astype(NPB)
    rhsb_full = rhsb_full.astype(NPB)

    def chunk_major(full):
        # [..., N] -> [NCHUNK, ..., CH]
        lead = full.shape[:-1]
        arr = full.reshape(*lead, NCHUNK, CH)
        perm = (len(lead),) + tuple(range(len(lead))) + (len(lead) + 1,)
        return np.ascontiguousarray(arr.transpose(perm))

    r8 = chunk_major(rhs8_full)    # [16, 128, 2, 512]
    rb = chunk_major(rhsb_full)    # [16, 66, 512]
    l8 = chunk_major(lhs8_full)
    lb = chunk_major(lhsb_full)

    in_maps = []
    for k in range(NCORE):
        order = [(k + d) % NCHUNK for d in range(NCHUNK)]
        in_maps.append({
            "lhs8": np.stack([l8[k], l8[(k + 8) % NCHUNK]]),
            "lhsb": np.stack([lb[k], lb[(k + 8) % NCHUNK]]),
            "rhs8": np.ascontiguousarray(r8[order]),
            "rhsb": np.ascontiguousarray(rb[order]),
        })
    return in_maps


def _combine(results):
    sgn = lambda ch: 1.0 if ch < NCHUNK // 2 else -1.0
    total = 0.0
    for k in range(NCORE):
        acc = np.asarray(results[k]["acc"], dtype=np.float64)  # [128, 17]
        colsum = acc.sum(axis=0)
        for col in range(NCOL):
            if col < 9:
                d, row_chunk = col, k
            else:
                d, row_chunk = col - 9, (k + 8) % NCHUNK
            col_chunk = (row_chunk + d) % NCHUNK
            w = 1.0 if d == 0 else 2.0
            s = sgn(row_chunk) * sgn(col_chunk)
            total += w * s * colsum[col]
    return total / (B * B)


def kernel(s0, s1, t0, t1):
    global LAST_EXEC_NS, LAST_RESULTS
    nc = _build()
    in_maps = _pack_inputs(
        np.asarray(s0), np.asarray(s1), np.asarray(t0), np.asarray(t1)
    )
    trace = os.environ.get("JMMD_TRACE", "0") == "1"
    res = run_bass_kernel_spmd(nc, in_maps, core_ids=list(range(NCORE)), trace=trace)
    LAST_EXEC_NS = res.exec_time_ns
    LAST_RESULTS = res
    return np.float32(_combine(res.results))
